# revision 66
# baseline (speedup 1.0000x reference)
"""Trainium2 Bass kernel for nn_CNN_Casual (LeNet-ish CNN, B=8192).

Pure data parallel over 8 NeuronCores: 1024 samples per core, parameters
replicated, one SPMD Bass program. Per core, samples are processed in
blocks of 128 (the TensorEngine stationary-operand width).

v2 design notes (vs the first working version):

  conv1  : host gathers x into overlapping windows (8 input rows x 16 cols
           = K 128) and folds sigmoid(mask) into a per-window Toeplitz
           weight matrix (exact - the mask is elementwise on the input and
           conv is linear). Columns are [d-block | b-block]: for each
           pooled column pair (a, b) the PE emits d = a - b and b directly
           (differenced Toeplitz columns - same column count as the raw
           conv).  max(a, b) = relu(d) + b, so the 2x2 max-pool becomes:
             ACT: relu(d)  (PSUM -> fp16 SBUF, was a plain copy before)
             DVE: m = relu_d + b  (one PSUM operand - hw allows only one)
             DVE: prp = max(m_row0, m_row1)   (fp16, 2x mode)
           which costs ~half of the reduce/copy-based drain of v1.
  T1     : PE transposes prp [128,120] slices into tpw PSUM fp16; ACT
           evicts relu(x + b1) to x2cat (bias is per-partition there).
  conv2  : 6 input-row Toeplitz planes per output-row-pair group; the
           first plane runs full width over the zero-padded master block
           with start=True (a start zeroes the whole PSUM bank, so only
           one per bank), the opposite edge plane accumulates N=160 -
           1760 PE cols/group instead of 1920.
  pool2  : DVE 6D reduce_max per group (single PSUM operand), PE T2
           transposes, ACT relu+bias evict into f_all.
  fc1/fc2: weights stationary [80,50] x4 / data stationary [50,128].
  softmax: constant-shift (fc2_b - 10 folded host-side) then ONE batched
           epilogue at the end of the core (Exp, windowed reduce_sum, Ln,
           subs) - a single activation-table load set covers
           Relu/Copy/Exp/Ln so table reloads are minimized.
  warmup : a couple of identity transposes after the identity DMA start
           the cost-model p-state ramp so conv1 runs at full clock.
  sched  : one merged 3-slot x 2-bank PSUM pool (conv1 pairs + conv2
           groups + fc accumulators) + a 2-buf fp16 transpose pool; the
           issue loop slides T1/conv2/evicts of block b-1 between the
           conv1 pairs of block b; fc per pair at odd iterations; the
           bulk softmax epilogue overlaps the last block's conv2 and only
           block 7's 10 columns chain at the tail.

dtypes: conv inputs/weights fp16 (PSUM accumulation fp32), pooled
activations fp16.  relu(d)+b rounds once more than a direct max, adding
<= ~1e-3 relative error on pooled values; end-to-end max rel err vs the
fp32 reference stays ~1e-3 (gate is 2e-2).

DMA: input is shipped pair-major [pair][128][12*256] so each per-pair DMA
is 128 descriptors of 6KB; weights ~1.9MB once; one output DMA.
"""

from contextlib import ExitStack

import numpy as np

import concourse.mybir as mybir
import concourse.tile as tile
from concourse import bacc
from concourse.bass_utils import run_bass_kernel_spmd

F32 = mybir.dt.float32
FP16 = mybir.dt.float16
AF = mybir.ActivationFunctionType
AX = mybir.AxisListType

N_CORES = 8
B_TOTAL = 8192
B_CORE = B_TOTAL // N_CORES  # 1024
N_PAIR = B_CORE // 256       # 4
N_BLK = B_CORE // 128        # 8


# --------------------------------------------------------------------------
# Host-side weight preparation (tiny tensors; exact rearrangement only)
# --------------------------------------------------------------------------
def _prep_weights(mask_w, conv1_w, conv1_b, conv2_w, conv2_b, fc1_w, fc1_b,
                  fc2_w, fc2_b):
    f32 = np.float32
    sig = (1.0 / (1.0 + np.exp(-mask_w.astype(f32)))).astype(f32)  # [28,28]
    w1 = conv1_w.astype(f32)[:, 0]  # [10,5,5]

    # conv1 Toeplitz, windows (w,h): input rows 4w..4w+7, cols 12h..12h+15.
    # K-row = i*16+j.  Columns: [d-block 240 | b-block 240], each ordered
    # (u,tr,o,m): raw out row p = 4w+2u+tr, raw col pair q = 12h+2m+{0,1};
    # d = raw(q even) - raw(q odd), b = raw(q odd); mask folded per tap.
    w1m = np.zeros((12, 128, 480), f32)
    for w in range(6):
        for h in range(2):
            t = 2 * w + h
            for u in range(2):
                for tr in range(2):
                    p = 4 * w + 2 * u + tr
                    for m in range(6):
                        cd = u * 120 + tr * 60 + m          # + o*6
                        cb = 240 + cd
                        for ki in range(5):
                            i = 2 * u + tr + ki
                            for kj in range(5):
                                j0 = 2 * m + kj
                                j1 = 2 * m + 1 + kj
                                wv = w1[:, ki, kj]  # [10]
                                s0 = sig[p + ki, 12 * h + j0]
                                s1 = sig[p + ki, 12 * h + j1]
                                oo = np.arange(10) * 6
                                w1m[t, i * 16 + j1, cb + oo] += wv * s1
                                w1m[t, i * 16 + j0, cd + oo] += wv * s0
                                w1m[t, i * 16 + j1, cd + oo] -= wv * s1
    w1m = np.ascontiguousarray(
        w1m.transpose(1, 0, 2).reshape(128, 5760)).astype(np.float16)

    # conv2 master Toeplitz: blocks [Z, W4, W3, W2, W1, W0, Z], each [120,160]
    # row index (c, j) = c*12 + j; col index (o2, q2) = o2*8 + q2
    w2m = np.zeros((120, 7, 160), np.float16)
    o2 = np.arange(20)
    for k in range(5):
        blk = 5 - k
        for c in range(10):
            for kj in range(5):
                for q2 in range(8):
                    j = q2 + kj
                    w2m[c * 12 + j, blk, o2 * 8 + q2] = conv2_w[:, c, k, kj]
    w2m_flat = np.ascontiguousarray(w2m.reshape(120, 7 * 160))
    # permute conv2 input rows from (c, j=h*6+m) to (h, c, m) so the T1
    # transpose reads contiguous prp2 blocks (see drain_pair)
    perm = np.empty(120, np.int64)
    for c in range(10):
        for h in range(2):
            for m in range(6):
                perm[h * 60 + c * 6 + m] = c * 12 + h * 6 + m
    w2m_flat = np.ascontiguousarray(w2m_flat[perm])

    # fc1 weights per pooled-row group p': rows (o2, s2), torch flatten order
    # of the conv2 activations is (o2, p', s2).
    fc1w4 = fc1_w.reshape(50, 20, 4, 4)  # [m, o2, p', s2]
    wfc1 = np.concatenate(
        [np.ascontiguousarray(fc1w4[:, :, p, :].reshape(50, 80).T)
         for p in range(4)],
        axis=1,
    )  # [80, 200]

    # const blob 1 (fp32): ident | bc2 | b1 | b2 | bf1  -> [128, 141]
    cst = np.zeros((128, 141), f32)
    cst[:, 0:128] = np.eye(128, dtype=f32)
    # constant stabilizing shift for log_softmax (exact: any per-sample
    # constant cancels); logits stay well inside fp32 exp range
    cst[:, 128:138] = np.tile(fc2_b.astype(f32).reshape(1, 10) - 10.0,
                              (128, 1))
    cst[0:120, 138] = np.tile(np.repeat(conv1_b.astype(f32), 6), 2)
    cst[0:80, 139] = np.repeat(conv2_b.astype(f32), 4)
    cst[0:50, 140] = fc1_b.astype(f32)

    # const blob 2 (fp16): fc2_w.T | wfc1 -> [80, 210]
    wfcb = np.zeros((80, 210), np.float16)
    wfcb[0:50, 0:10] = fc2_w.T.astype(np.float16)
    wfcb[:, 10:210] = wfc1.astype(np.float16)

    idb = np.eye(128).astype(np.float16)
    return dict(w1m=w1m, w2m=w2m_flat, wfcb=wfcb, cst=cst, idb=idb)


# --------------------------------------------------------------------------
# Device program
# --------------------------------------------------------------------------
def _build(b_core):
    assert b_core == 1024
    n_pair = N_PAIR

    nc = bacc.Bacc("TRN2", target_bir_lowering=False, debug=False,
                   num_devices=N_CORES)

    xw_d = nc.dram_tensor("xw", [n_pair, 128, 3072], FP16,
                          kind="ExternalInput").ap()
    w1m_d = nc.dram_tensor("w1m", [128, 5760], FP16,
                           kind="ExternalInput").ap()
    w2m_d = nc.dram_tensor("w2m", [120, 1120], FP16, kind="ExternalInput").ap()
    wfcb_d = nc.dram_tensor("wfcb", [80, 210], FP16, kind="ExternalInput").ap()
    cst_d = nc.dram_tensor("cst", [128, 141], F32, kind="ExternalInput").ap()
    idb_d = nc.dram_tensor("idb", [128, 128], FP16, kind="ExternalInput").ap()
    y = nc.dram_tensor("y", [b_core, 10], F32, kind="ExternalOutput").ap()

    with tile.TileContext(nc) as tc, ExitStack() as ctx:
        consts = ctx.enter_context(tc.tile_pool(name="consts", bufs=1))
        identb = consts.tile([128, 128], FP16)
        w1m_sb = consts.tile([128, 5760], FP16)
        w2m_sb = consts.tile([120, 1120], FP16)
        wfcb_sb = consts.tile([80, 210], FP16)
        cst_sb = consts.tile([128, 141], F32)
        t1_all = consts.tile([128, 10 * N_BLK], F32)

        bc2_sb = cst_sb[:, 128:138]
        b1_sb = cst_sb[0:120, 138:139]
        b2_sb = cst_sb[0:80, 139:140]
        bf1_sb = cst_sb[0:50, 140:141]
        wfc2_sb = wfcb_sb[0:50, 0:10]
        wfc1_sb = wfcb_sb[:, 10:210]

        xw_pool = ctx.enter_context(tc.tile_pool(name="xw", bufs=2))
        # one merged 3-slot x 2-bank PSUM pool for conv1 pairs, conv2
        # groups, and fc accumulators; tp (fp16 transpose staging) separate
        psx_pool = ctx.enter_context(tc.tile_pool(name="psx", bufs=3,
                                                  space="PSUM"))
        rd_pool = ctx.enter_context(tc.tile_pool(name="rd", bufs=6))
        mp_pool = ctx.enter_context(tc.tile_pool(name="mp", bufs=6))
        prp_pool = ctx.enter_context(tc.tile_pool(name="prp", bufs=14))
        tp_pool = ctx.enter_context(tc.tile_pool(name="tp", bufs=2,
                                                 space="PSUM"))
        x2_pool = ctx.enter_context(tc.tile_pool(name="x2", bufs=6))
        p2_pool = ctx.enter_context(tc.tile_pool(name="p2", bufs=6))
        f_pool = ctx.enter_context(tc.tile_pool(name="fp", bufs=2))
        fc1o_pool = ctx.enter_context(tc.tile_pool(name="fc1o", bufs=2))
        sm_pool = ctx.enter_context(tc.tile_pool(name="sm", bufs=1))

        # ---- PE warmup: ramp the p-state while DMAs land ----
        warm = psx_pool.tile([128, 128], FP16, name="warm", tag="psx")
        for _ in range(2):
            nc.tensor.transpose(warm[:], identb[:], identb[:])
        # pre-load the activation-table set covering Relu/Copy/Exp/Ln so the
        # fixpoint pass inserts no further loads (set 6 =
        # natural_log_exp_and_others in act_info.json insertion order)
        nc.scalar.add_instruction(mybir.InstLoadActFuncSet(
            name=nc.get_next_instruction_name(), act_func_set_id=6,
            ins=[], outs=[]))

        # initial weight DMAs: xw on SP/HWDGE, weights on gpsimd/SWDGE so
        # the two issue paths run in parallel at startup
        xw_tiles = [None] * n_pair

        def issue_xw(pair, defer=False):
            xwc = xw_pool.tile([128, 3072], FP16, name=f"xwc{pair}", tag="xw")
            xw_tiles[pair] = xwc
            if not defer:
                nc.sync.dma_start(xwc[:], xw_d[pair])
            return xwc

        xw_tiles_first = issue_xw(0, defer=True)

        nc.sync.dma_start(xw_tiles_first[:, 0:1536], xw_d[0, :, 0:1536])
        nc.sync.dma_start(identb[:], idb_d)
        for q in range(3):
            nc.sync.dma_start(w1m_sb[:, q * 960:(q + 1) * 960],
                              w1m_d[:, q * 960:(q + 1) * 960])
        nc.sync.dma_start(xw_tiles_first[:, 1536:3072], xw_d[0, :, 1536:3072])
        for q in range(3, 6):
            nc.sync.dma_start(w1m_sb[:, q * 960:(q + 1) * 960],
                              w1m_d[:, q * 960:(q + 1) * 960])

        # per-block state
        state = {}

        def conv1_pair(b, w):
            """Two window matmuls (t=2w, 2w+1) into one 2-bank PSUM tile.

            High priority: the PE should always prefer feeding the drain
            pipeline; conv2/transposes of the previous block fill the gaps.
            """
            pair, half = b // 2, b % 2
            ps1p = psx_pool.tile([128, 1024], F32, name="ps1_t", tag="psx")
            with tc.high_priority(offset=30):
                for h in range(2):
                    t = 2 * w + h
                    nc.tensor.matmul(ps1p[:, h * 512:h * 512 + 480],
                                     xw_tiles[pair][:, t * 256 + half * 128:
                                                    t * 256 + half * 128
                                                    + 128],
                                     w1m_sb[:, t * 480:(t + 1) * 480],
                                     start=True, stop=True)
            return ps1p

        def drain_pair(b, w, ps1p):
            """relu(d) on ACT, += b on DVE; level-2 max batched per two
            window-pairs into a [128, 480] prp2 tile."""
            st = state[b]
            psv = ps1p.rearrange("p (h q) -> p h q", h=2)
            rd = rd_pool.tile([128, 480], FP16, name="rd_t", tag="rd")
            if w % 2 == 0:
                st["mp"] = mp_pool.tile([128, 960], FP16, name="mp_t",
                                        tag="mp")
            m_sup = st["mp"]
            # m_sup layout (P, u, h, tr, o, m): the level-2 group dim
            # (P,u,h) then matches prp2's contiguous (P,u,h,o,m) layout
            m_pair = m_sup[:, (w % 2) * 480:(w % 2) * 480 + 480]
            nc.scalar.activation(rd.rearrange("p (h c) -> p h c", h=2),
                                 psv[:, :, 0:240], AF.Relu)
            nc.vector.tensor_add(
                m_pair.rearrange("p (u h c) -> p h u c", u=2, h=2),
                rd.rearrange("p (h u c) -> p h u c", h=2, u=2),
                psv[:, :, 240:480].rearrange("p h (u c) -> p h u c", u=2))
            if w % 2 == 1:
                # prp2 contiguous layout (P, h, u, o, m): both tensor_max
                # APs merge to <=2 free dims (walrus limit); the T1
                # transpose reads a strided (o, h, m) view instead.
                prp2 = prp_pool.tile([128, 480], FP16, name="prp_t",
                                     tag="prp")
                st["prp"].append(prp2)
                mv = m_sup.rearrange("p (g tr om) -> p g tr om",
                                     g=8, tr=2)
                pv = prp2.rearrange("p (g om) -> p g om", g=8)
                nc.vector.tensor_max(pv, mv[:, :, 0], mv[:, :, 1])

        def t1_transposes(b):
            """PE transposes prp2 slices -> two 6-row tpw tiles [120, 768].

            prp2 tile P holds windows 2P (cols 0:240) and 2P+1 (240:480).
            """
            st = state[b]
            st["tpw"] = []
            st["x2"] = []
            for ww in range(2):
                tpw = tp_pool.tile([120, 768], FP16, name="tpw_t", tag="tp")
                st["tpw"].append(tpw)
                for k in range(3):
                    w = ww * 3 + k
                    prp2 = st["prp"][w // 2]
                    base = (w % 2) * 240
                    for u in range(2):
                        nc.tensor.transpose(
                            tpw[:, (k * 2 + u) * 128:(k * 2 + u + 1) * 128],
                            prp2[:, base + u * 120:base + u * 120 + 120],
                            identb[:])

        def x2c_evict(b, ww):
            """ACT relu+bias evict of one 6-row tpw tile -> x2cat."""
            st = state[b]
            x2c = x2_pool.tile([120, 768], FP16, name="x2c_t", tag=f"x2c{ww}")
            nc.scalar.activation(x2c[:], st["tpw"][ww][:], AF.Relu,
                                 bias=b1_sb[:, 0:1])
            st["x2"].append(x2c)

        def conv2_group(b, g):
            """6 Toeplitz planes (edge planes N=160), pool2 reduce, T2."""
            st = state[b]
            x2cat = st["x2"]
            if g == 0:
                st["tp2"] = tp_pool.tile([80, 512], FP16, name="tp2w_t",
                                         tag="tp")
            tp2w = st["tp2"]
            ps2g = psx_pool.tile([128, 320], F32, name=f"ps2_{g}", tag="psx")

            def lhsT(r):
                return x2cat[r // 6][:, (r % 6) * 128:(r % 6 + 1) * 128]

            # d=0 runs full width [W0|Z] with start=True (start zeroes the
            # whole bank, so only ONE start per bank); d=5 edge accumulates
            # N=160 into the already-zeroed upper half
            nc.tensor.matmul(ps2g[:], lhsT(2 * g),
                             w2m_sb[:, 800:1120], start=True, stop=False,
                             skip_group_check=True)
            nc.tensor.matmul(ps2g[:, 160:320], lhsT(2 * g + 5),
                             w2m_sb[:, 160:320], start=False, stop=False,
                             skip_group_check=True)
            for d in range(1, 5):
                nc.tensor.matmul(ps2g[:], lhsT(2 * g + d),
                                 w2m_sb[:, (5 - d) * 160:(7 - d) * 160],
                                 start=False, stop=(d == 4),
                                 skip_group_check=True)
            p2 = p2_pool.tile([128, 80], FP16, name="p2_t", tag="p2")
            p2v = p2.rearrange("p (o s) -> p o s", o=20)
            src = ps2g.rearrange("p (pl o s tc) -> p o s pl tc",
                                 pl=2, o=20, s=4)
            nc.vector.reduce_max(p2v, src, axis=AX.XY)
            # defer the T2 transpose to the next group so the PE is not
            # blocked on this group's pool2 reduce
            if g > 0:
                nc.tensor.transpose(tp2w[:, (g - 1) * 128:g * 128],
                                    st["p2"], identb[:])
            st["p2"] = p2

        def f_evict(b):
            pair, half = b // 2, b % 2
            st = state[b]
            nc.tensor.transpose(st["tp2"][:, 384:512], st["p2"], identb[:])
            if half == 0:
                st["f"] = f_pool.tile([80, 1024], FP16, name="f_all",
                                      tag="f_all")
            else:
                st["f"] = state[b - 1]["f"]
            nc.scalar.activation(st["f"][:, half * 512:half * 512 + 512],
                                 st["tp2"][:], AF.Relu, bias=b2_sb[:, 0:1])

        def fc_pair(pair):
            """Batched fc1+fc2 for both blocks of a pair."""
            f_all = state[pair * 2 + 1]["f"]
            fview = f_all.rearrange("p (h g n) -> p g h n", h=2, g=4, n=128)
            psf1 = psx_pool.tile([50, 256], F32, name="psf1", tag="psx")
            for g in range(4):
                nc.tensor.matmul(psf1[:], wfc1_sb[:, g * 50:(g + 1) * 50],
                                 fview[:, g], start=(g == 0), stop=(g == 3))
            fc1o = fc1o_pool.tile([50, 256], FP16, name="fc1o", tag="fc1o")
            nc.scalar.activation(fc1o[:], psf1[:], AF.Relu,
                                 bias=bf1_sb[:, 0:1])
            for half in range(2):
                blk = pair * 2 + half
                psf2 = psx_pool.tile([128, 10], F32, name="psf2", tag="psx")
                nc.tensor.matmul(psf2[:],
                                 fc1o[:, half * 128:half * 128 + 128],
                                 wfc2_sb[:], start=True, stop=True)
                nc.vector.tensor_add(t1_all[:, blk * 10:blk * 10 + 10],
                                     psf2[:], bc2_sb[:])

        def fc_half(blk):
            """fc1+fc2 for one 128-sample block (independent sample half of
            the pair's f_all) - runs right after that block's f_evict."""
            half = blk % 2
            f_all = state[blk]["f"]
            fview = f_all.rearrange("p (h g n) -> p g h n", h=2, g=4, n=128)
            psf1 = psx_pool.tile([50, 128], F32, name="psf1", tag="psx")
            for g in range(4):
                nc.tensor.matmul(psf1[:], wfc1_sb[:, g * 50:(g + 1) * 50],
                                 fview[:, g, half], start=(g == 0),
                                 stop=(g == 3))
            fc1o = fc1o_pool.tile([50, 128], FP16, name="fc1o", tag="fc1o")
            nc.scalar.activation(fc1o[:], psf1[:], AF.Relu,
                                 bias=bf1_sb[:, 0:1])
            psf2 = psx_pool.tile([128, 10], F32, name="psf2", tag="psx")
            nc.tensor.matmul(psf2[:], fc1o[:], wfc2_sb[:],
                             start=True, stop=True)
            nc.vector.tensor_add(t1_all[:, blk * 10:blk * 10 + 10],
                                 psf2[:], bc2_sb[:])

        # ---------------- main software-pipelined issue loop --------------
        # iteration b: conv1 pairs of block b interleaved with conv2 groups
        # of block b-1 on the PE; ACT drains relu_d(b) between the x2cat
        # evicts of block b-1; DVE adds/maxes (b) between pool2 reduces of
        # b-1.  fc of pair p issues mid-iteration 2p+2.
        def epilogue_q(b0, nb, tag):
            """log_softmax for blocks b0 .. b0+nb-1 and their y DMA."""
            t1s = t1_all[:, b0 * 10:(b0 + nb) * 10]
            e4 = sm_pool.tile([128, 10 * nb], F32, name=f"e4_{tag}",
                              tag=f"e{tag}")
            nc.scalar.activation(e4[:], t1s, AF.Exp)
            se = sm_pool.tile([128, nb], F32, name=f"se_{tag}",
                              tag=f"se{tag}")
            nc.vector.reduce_sum(
                se[:], e4.rearrange("p (b t) -> p b t", t=10), axis=AX.X)
            ls = sm_pool.tile([128, nb], F32, name=f"ls_{tag}",
                              tag=f"ls{tag}")
            nc.scalar.activation(ls[:], se[:], AF.Ln)
            yo = sm_pool.tile([128, 10 * nb], F32, name=f"yo_{tag}",
                              tag=f"yo{tag}")
            for b in range(nb):
                nc.vector.tensor_scalar_sub(
                    yo[:, b * 10:b * 10 + 10],
                    t1s[:, b * 10:b * 10 + 10],
                    ls[:, b:b + 1])
            nc.sync.dma_start(
                y[b0 * 128:(b0 + nb) * 128]
                .rearrange("(blk p) c -> p blk c", p=128),
                yo.rearrange("p (blk c) -> p blk c", c=10))

        # iteration b issues conv1 pairs of block b; T1/conv2/evicts of
        # block b-1 slide into it as PE/ACT filler between the pairs.
        for b in range(N_BLK + 1):
            prev = b - 1
            if b < N_BLK:
                state[b] = {"prp": []}
                if b % 2 == 1 and b // 2 + 1 < n_pair:
                    issue_xw(b // 2 + 1)
                for w in range(6):
                    ps1p = conv1_pair(b, w)
                    if b == 0:
                        if w == 1:
                            nc.sync.dma_start(cst_sb[:], cst_d)
                        elif w == 2:
                            nc.sync.dma_start(w2m_sb[:], w2m_d)
                        elif w == 3:
                            nc.sync.dma_start(wfcb_sb[:], wfcb_d)
                        for _ in range(5):
                            nc.tensor.transpose(warm[:], identb[:],
                                                identb[:])
                    drain_pair(b, w, ps1p)
                    if prev >= 0:
                        if w == 1:
                            t1_transposes(prev)
                        elif w == 2:
                            x2c_evict(prev, 0)
                            conv2_group(prev, 0)
                        elif w == 3:
                            x2c_evict(prev, 1)
                            conv2_group(prev, 1)
                        elif w == 4:
                            conv2_group(prev, 2)
                        elif w == 5:
                            conv2_group(prev, 3)
                if prev >= 0:
                    f_evict(prev)
                    if b % 2 == 1 and b >= 3:
                        fc_pair((b - 3) // 2)

            else:
                fc_half(prev - 1)
                epilogue_q(0, 7, "a")
                t1_transposes(prev)
                x2c_evict(prev, 0)
                conv2_group(prev, 0)
                x2c_evict(prev, 1)
                for g in range(1, 4):
                    conv2_group(prev, g)
                f_evict(prev)
                fc_half(prev)
                epilogue_q(7, 1, "b")

    nc.compile()
    return nc


_PROGRAM_CACHE = {}


def _get_program(b_core):
    if b_core not in _PROGRAM_CACHE:
        _PROGRAM_CACHE[b_core] = _build(b_core)
    return _PROGRAM_CACHE[b_core]


def make_in_maps(x, weights, b_core=B_CORE, n_cores=N_CORES):
    """Shard x over cores; replicate the (rearranged) parameters."""
    f32 = np.float32
    xr = np.asarray(x, dtype=f32).reshape(-1, 28, 28)
    in_maps = []
    for c in range(n_cores):
        xc = xr[c * b_core:(c + 1) * b_core]  # [b_core, 28, 28]
        xwin = np.empty((12, 128, b_core), np.float16)
        for w in range(6):
            for h in range(2):
                win = xc[:, 4 * w:4 * w + 8, 12 * h:12 * h + 16]
                xwin[w * 2 + h] = win.reshape(b_core, 128).T
        # pair-major, partition-major: [pair][128][t*256+n]
        xp = xwin.reshape(12, 128, N_PAIR, 256).transpose(2, 1, 0, 3)
        m = {"xw": np.ascontiguousarray(xp.reshape(N_PAIR, 128, 3072))}
        m.update(weights)
        in_maps.append(m)
    return in_maps


def kernel(**inputs):
    x = np.asarray(inputs["x"], dtype=np.float32)
    weights = _prep_weights(
        np.asarray(inputs["mask_w"], np.float32),
        np.asarray(inputs["conv1_w"], np.float32),
        np.asarray(inputs["conv1_b"], np.float32),
        np.asarray(inputs["conv2_w"], np.float32),
        np.asarray(inputs["conv2_b"], np.float32),
        np.asarray(inputs["fc1_w"], np.float32),
        np.asarray(inputs["fc1_b"], np.float32),
        np.asarray(inputs["fc2_w"], np.float32),
        np.asarray(inputs["fc2_b"], np.float32),
    )
    nc = _get_program(B_CORE)
    in_maps = make_in_maps(x, weights)
    res = run_bass_kernel_spmd(nc, in_maps, list(range(N_CORES)))
    out = np.concatenate([res.results[c]["y"] for c in range(N_CORES)], axis=0)
    return np.ascontiguousarray(out.astype(np.float32))


if __name__ == "__main__":
    rng = np.random.default_rng(0)
    ins = {
        "x": rng.standard_normal((B_TOTAL, 1, 28, 28), dtype=np.float32),
        "mask_w": rng.standard_normal((28, 28), dtype=np.float32) * 0.1,
        "conv1_w": rng.standard_normal((10, 1, 5, 5), dtype=np.float32) * 0.2,
        "conv1_b": rng.standard_normal((10,), dtype=np.float32) * 0.1,
        "conv2_w": rng.standard_normal((20, 10, 5, 5), dtype=np.float32) * 0.06,
        "conv2_b": rng.standard_normal((20,), dtype=np.float32) * 0.1,
        "fc1_w": rng.standard_normal((50, 320), dtype=np.float32) * 0.05,
        "fc1_b": rng.standard_normal((50,), dtype=np.float32) * 0.1,
        "fc2_w": rng.standard_normal((10, 50), dtype=np.float32) * 0.14,
        "fc2_b": rng.standard_normal((10,), dtype=np.float32) * 0.1,
    }
    out = kernel(**ins)
    print(out.shape, out.dtype, out[:2])


# revision 70
# speedup vs baseline: 1.0055x; 1.0055x over previous
"""Trainium2 Bass kernel for nn_CNN_Casual (LeNet-ish CNN, B=8192).

Pure data parallel over 8 NeuronCores: 1024 samples per core, parameters
replicated, one SPMD Bass program. Per core, samples are processed in
blocks of 128 (the TensorEngine stationary-operand width).

v2 design notes (vs the first working version):

  conv1  : host gathers x into overlapping windows (8 input rows x 16 cols
           = K 128) and folds sigmoid(mask) into a per-window Toeplitz
           weight matrix (exact - the mask is elementwise on the input and
           conv is linear). Columns are [d-block | b-block]: for each
           pooled column pair (a, b) the PE emits d = a - b and b directly
           (differenced Toeplitz columns - same column count as the raw
           conv).  max(a, b) = relu(d) + b, so the 2x2 max-pool becomes:
             ACT: relu(d)  (PSUM -> fp16 SBUF, was a plain copy before)
             DVE: m = relu_d + b  (one PSUM operand - hw allows only one)
             DVE: prp = max(m_row0, m_row1)   (fp16, 2x mode)
           which costs ~half of the reduce/copy-based drain of v1.
  T1     : PE transposes prp [128,120] slices into tpw PSUM fp16; ACT
           evicts relu(x + b1) to x2cat (bias is per-partition there).
  conv2  : 6 input-row Toeplitz planes per output-row-pair group; the
           first plane runs full width over the zero-padded master block
           with start=True (a start zeroes the whole PSUM bank, so only
           one per bank), the opposite edge plane accumulates N=160 -
           1760 PE cols/group instead of 1920.
  pool2  : DVE 6D reduce_max per group (single PSUM operand), PE T2
           transposes, ACT relu+bias evict into f_all.
  fc1/fc2: weights stationary [80,50] x4 / data stationary [50,128].
  softmax: constant-shift (fc2_b - 10 folded host-side) then ONE batched
           epilogue at the end of the core (Exp, windowed reduce_sum, Ln,
           subs) - a single activation-table load set covers
           Relu/Copy/Exp/Ln so table reloads are minimized.
  warmup : a couple of identity transposes after the identity DMA start
           the cost-model p-state ramp so conv1 runs at full clock.
  sched  : one merged 3-slot x 2-bank PSUM pool (conv1 pairs + conv2
           groups + fc accumulators) + a 2-buf fp16 transpose pool; the
           issue loop slides T1/conv2/evicts of block b-1 between the
           conv1 pairs of block b; fc per pair at odd iterations; the
           bulk softmax epilogue overlaps the last block's conv2 and only
           block 7's 10 columns chain at the tail.

dtypes: conv inputs/weights fp16 (PSUM accumulation fp32), pooled
activations fp16.  relu(d)+b rounds once more than a direct max, adding
<= ~1e-3 relative error on pooled values; end-to-end max rel err vs the
fp32 reference stays ~1e-3 (gate is 2e-2).

DMA: input is shipped pair-major [pair][128][12*256] so each per-pair DMA
is 128 descriptors of 6KB; weights ~1.9MB once; one output DMA.
"""

from contextlib import ExitStack

import numpy as np

import concourse.mybir as mybir
import concourse.tile as tile
from concourse import bacc
from concourse.bass_utils import run_bass_kernel_spmd

F32 = mybir.dt.float32
FP16 = mybir.dt.float16
AF = mybir.ActivationFunctionType
AX = mybir.AxisListType

N_CORES = 8
B_TOTAL = 8192
B_CORE = B_TOTAL // N_CORES  # 1024
N_PAIR = B_CORE // 256       # 4
N_BLK = B_CORE // 128        # 8


# --------------------------------------------------------------------------
# Host-side weight preparation (tiny tensors; exact rearrangement only)
# --------------------------------------------------------------------------
def _prep_weights(mask_w, conv1_w, conv1_b, conv2_w, conv2_b, fc1_w, fc1_b,
                  fc2_w, fc2_b):
    f32 = np.float32
    sig = (1.0 / (1.0 + np.exp(-mask_w.astype(f32)))).astype(f32)  # [28,28]
    w1 = conv1_w.astype(f32)[:, 0]  # [10,5,5]

    # conv1 Toeplitz, windows (w,h): input rows 4w..4w+7, cols 12h..12h+15.
    # K-row = i*16+j.  Columns: [d-block 240 | b-block 240], each ordered
    # (u,tr,o,m): raw out row p = 4w+2u+tr, raw col pair q = 12h+2m+{0,1};
    # d = raw(q even) - raw(q odd), b = raw(q odd); mask folded per tap.
    w1m = np.zeros((12, 128, 480), f32)
    for w in range(6):
        for h in range(2):
            t = 2 * w + h
            for u in range(2):
                for tr in range(2):
                    p = 4 * w + 2 * u + tr
                    for m in range(6):
                        cd = u * 120 + tr * 60 + m          # + o*6
                        cb = 240 + cd
                        for ki in range(5):
                            i = 2 * u + tr + ki
                            for kj in range(5):
                                j0 = 2 * m + kj
                                j1 = 2 * m + 1 + kj
                                wv = w1[:, ki, kj]  # [10]
                                s0 = sig[p + ki, 12 * h + j0]
                                s1 = sig[p + ki, 12 * h + j1]
                                oo = np.arange(10) * 6
                                w1m[t, i * 16 + j1, cb + oo] += wv * s1
                                w1m[t, i * 16 + j0, cd + oo] += wv * s0
                                w1m[t, i * 16 + j1, cd + oo] -= wv * s1
    w1m = np.ascontiguousarray(
        w1m.transpose(1, 0, 2).reshape(128, 5760)).astype(np.float16)

    # conv2 master Toeplitz: blocks [Z, W4, W3, W2, W1, W0, Z], each [120,160]
    # row index (c, j) = c*12 + j; col index (o2, q2) = o2*8 + q2
    w2m = np.zeros((120, 7, 160), np.float16)
    o2 = np.arange(20)
    for k in range(5):
        blk = 5 - k
        for c in range(10):
            for kj in range(5):
                for q2 in range(8):
                    j = q2 + kj
                    w2m[c * 12 + j, blk, o2 * 8 + q2] = conv2_w[:, c, k, kj]
    w2m_flat = np.ascontiguousarray(w2m.reshape(120, 7 * 160))
    # permute conv2 input rows from (c, j=h*6+m) to (h, c, m) so the T1
    # transpose reads contiguous prp2 blocks (see drain_pair)
    perm = np.empty(120, np.int64)
    for c in range(10):
        for h in range(2):
            for m in range(6):
                perm[h * 60 + c * 6 + m] = c * 12 + h * 6 + m
    w2m_flat = np.ascontiguousarray(w2m_flat[perm])

    # fc1 weights per pooled-row group p': rows (o2, s2), torch flatten order
    # of the conv2 activations is (o2, p', s2).
    fc1w4 = fc1_w.reshape(50, 20, 4, 4)  # [m, o2, p', s2]
    wfc1 = np.concatenate(
        [np.ascontiguousarray(fc1w4[:, :, p, :].reshape(50, 80).T)
         for p in range(4)],
        axis=1,
    )  # [80, 200]

    # const blob 1 (fp32): ident | bc2 | b1 | b2 | bf1  -> [128, 141]
    cst = np.zeros((128, 141), f32)
    cst[:, 0:128] = np.eye(128, dtype=f32)
    # constant stabilizing shift for log_softmax (exact: any per-sample
    # constant cancels); logits stay well inside fp32 exp range
    cst[:, 128:138] = np.tile(fc2_b.astype(f32).reshape(1, 10) - 10.0,
                              (128, 1))
    cst[0:120, 138] = np.tile(np.repeat(conv1_b.astype(f32), 6), 2)
    cst[0:80, 139] = np.repeat(conv2_b.astype(f32), 4)
    cst[0:50, 140] = fc1_b.astype(f32)

    # const blob 2 (fp16): fc2_w.T | wfc1 -> [80, 210]
    wfcb = np.zeros((80, 210), np.float16)
    wfcb[0:50, 0:10] = fc2_w.T.astype(np.float16)
    wfcb[:, 10:210] = wfc1.astype(np.float16)

    idb = np.eye(128).astype(np.float16)
    return dict(w1m=w1m, w2m=w2m_flat, wfcb=wfcb, cst=cst, idb=idb)


# --------------------------------------------------------------------------
# Device program
# --------------------------------------------------------------------------
def _build(b_core):
    assert b_core == 1024
    n_pair = N_PAIR

    nc = bacc.Bacc("TRN2", target_bir_lowering=False, debug=False,
                   num_devices=N_CORES)

    xw_d = nc.dram_tensor("xw", [n_pair, 128, 3072], FP16,
                          kind="ExternalInput").ap()
    w1m_d = nc.dram_tensor("w1m", [128, 5760], FP16,
                           kind="ExternalInput").ap()
    w2m_d = nc.dram_tensor("w2m", [120, 1120], FP16, kind="ExternalInput").ap()
    wfcb_d = nc.dram_tensor("wfcb", [80, 210], FP16, kind="ExternalInput").ap()
    cst_d = nc.dram_tensor("cst", [128, 141], F32, kind="ExternalInput").ap()
    idb_d = nc.dram_tensor("idb", [128, 128], FP16, kind="ExternalInput").ap()
    y = nc.dram_tensor("y", [b_core, 10], F32, kind="ExternalOutput").ap()

    with tile.TileContext(nc) as tc, ExitStack() as ctx:
        consts = ctx.enter_context(tc.tile_pool(name="consts", bufs=1))
        identb = consts.tile([128, 128], FP16)
        w1m_sb = consts.tile([128, 5760], FP16)
        w2m_sb = consts.tile([120, 1120], FP16)
        wfcb_sb = consts.tile([80, 210], FP16)
        cst_sb = consts.tile([128, 141], F32)
        t1_all = consts.tile([128, 10 * N_BLK], F32)

        bc2_sb = cst_sb[:, 128:138]
        b1_sb = cst_sb[0:120, 138:139]
        b2_sb = cst_sb[0:80, 139:140]
        bf1_sb = cst_sb[0:50, 140:141]
        wfc2_sb = wfcb_sb[0:50, 0:10]
        wfc1_sb = wfcb_sb[:, 10:210]

        xw_pool = ctx.enter_context(tc.tile_pool(name="xw", bufs=2))
        # one merged 3-slot x 2-bank PSUM pool for conv1 pairs, conv2
        # groups, and fc accumulators; tp (fp16 transpose staging) separate
        psx_pool = ctx.enter_context(tc.tile_pool(name="psx", bufs=3,
                                                  space="PSUM"))
        rd_pool = ctx.enter_context(tc.tile_pool(name="rd", bufs=6))
        mp_pool = ctx.enter_context(tc.tile_pool(name="mp", bufs=6))
        prp_pool = ctx.enter_context(tc.tile_pool(name="prp", bufs=14))
        tp_pool = ctx.enter_context(tc.tile_pool(name="tp", bufs=2,
                                                 space="PSUM"))
        x2_pool = ctx.enter_context(tc.tile_pool(name="x2", bufs=6))
        p2_pool = ctx.enter_context(tc.tile_pool(name="p2", bufs=6))
        f_pool = ctx.enter_context(tc.tile_pool(name="fp", bufs=2))
        fc1o_pool = ctx.enter_context(tc.tile_pool(name="fc1o", bufs=2))
        sm_pool = ctx.enter_context(tc.tile_pool(name="sm", bufs=1))

        # ---- PE warmup: ramp the p-state while DMAs land ----
        warm = psx_pool.tile([128, 128], FP16, name="warm", tag="psx")
        for _ in range(4):
            nc.tensor.transpose(warm[:], identb[:], identb[:])
        # pre-load the activation-table set covering Relu/Copy/Exp/Ln so the
        # fixpoint pass inserts no further loads (set 6 =
        # natural_log_exp_and_others in act_info.json insertion order)
        nc.scalar.add_instruction(mybir.InstLoadActFuncSet(
            name=nc.get_next_instruction_name(), act_func_set_id=6,
            ins=[], outs=[]))

        # initial weight DMAs: xw on SP/HWDGE, weights on gpsimd/SWDGE so
        # the two issue paths run in parallel at startup
        xw_tiles = [None] * n_pair

        def issue_xw(pair, defer=False):
            xwc = xw_pool.tile([128, 3072], FP16, name=f"xwc{pair}", tag="xw")
            xw_tiles[pair] = xwc
            if not defer:
                nc.sync.dma_start(xwc[:], xw_d[pair])
            return xwc

        xw_tiles_first = issue_xw(0, defer=True)

        nc.sync.dma_start(xw_tiles_first[:, 0:1536], xw_d[0, :, 0:1536])
        nc.sync.dma_start(identb[:], idb_d)
        for q in range(3):
            nc.sync.dma_start(w1m_sb[:, q * 960:(q + 1) * 960],
                              w1m_d[:, q * 960:(q + 1) * 960])
        nc.sync.dma_start(xw_tiles_first[:, 1536:3072], xw_d[0, :, 1536:3072])
        for q in range(3, 6):
            nc.sync.dma_start(w1m_sb[:, q * 960:(q + 1) * 960],
                              w1m_d[:, q * 960:(q + 1) * 960])

        # per-block state
        state = {}

        def conv1_pair(b, w):
            """Two window matmuls (t=2w, 2w+1) into one 2-bank PSUM tile.

            High priority: the PE should always prefer feeding the drain
            pipeline; conv2/transposes of the previous block fill the gaps.
            """
            pair, half = b // 2, b % 2
            ps1p = psx_pool.tile([128, 1024], F32, name="ps1_t", tag="psx")
            with tc.high_priority(offset=30):
                for h in range(2):
                    t = 2 * w + h
                    nc.tensor.matmul(ps1p[:, h * 512:h * 512 + 480],
                                     xw_tiles[pair][:, t * 256 + half * 128:
                                                    t * 256 + half * 128
                                                    + 128],
                                     w1m_sb[:, t * 480:(t + 1) * 480],
                                     start=True, stop=True)
            return ps1p

        def drain_pair(b, w, ps1p):
            """relu(d) on ACT, += b on DVE; level-2 max batched per two
            window-pairs into a [128, 480] prp2 tile."""
            st = state[b]
            psv = ps1p.rearrange("p (h q) -> p h q", h=2)
            rd = rd_pool.tile([128, 480], FP16, name="rd_t", tag="rd")
            if w % 2 == 0:
                st["mp"] = mp_pool.tile([128, 960], FP16, name="mp_t",
                                        tag="mp")
            m_sup = st["mp"]
            # m_sup layout (P, u, h, tr, o, m): the level-2 group dim
            # (P,u,h) then matches prp2's contiguous (P,u,h,o,m) layout
            m_pair = m_sup[:, (w % 2) * 480:(w % 2) * 480 + 480]
            nc.scalar.activation(rd.rearrange("p (h c) -> p h c", h=2),
                                 psv[:, :, 0:240], AF.Relu)
            nc.vector.tensor_add(
                m_pair.rearrange("p (u h c) -> p h u c", u=2, h=2),
                rd.rearrange("p (h u c) -> p h u c", h=2, u=2),
                psv[:, :, 240:480].rearrange("p h (u c) -> p h u c", u=2))
            if w % 2 == 1:
                # prp2 contiguous layout (P, h, u, o, m): both tensor_max
                # APs merge to <=2 free dims (walrus limit); the T1
                # transpose reads a strided (o, h, m) view instead.
                prp2 = prp_pool.tile([128, 480], FP16, name="prp_t",
                                     tag="prp")
                st["prp"].append(prp2)
                mv = m_sup.rearrange("p (g tr om) -> p g tr om",
                                     g=8, tr=2)
                pv = prp2.rearrange("p (g om) -> p g om", g=8)
                nc.vector.tensor_max(pv, mv[:, :, 0], mv[:, :, 1])

        def t1_transposes(b):
            """PE transposes prp2 slices -> two 6-row tpw tiles [120, 768].

            prp2 tile P holds windows 2P (cols 0:240) and 2P+1 (240:480).
            """
            st = state[b]
            st["tpw"] = []
            st["x2"] = []
            for ww in range(2):
                tpw = tp_pool.tile([120, 768], FP16, name="tpw_t", tag="tp")
                st["tpw"].append(tpw)
                for k in range(3):
                    w = ww * 3 + k
                    prp2 = st["prp"][w // 2]
                    base = (w % 2) * 240
                    for u in range(2):
                        nc.tensor.transpose(
                            tpw[:, (k * 2 + u) * 128:(k * 2 + u + 1) * 128],
                            prp2[:, base + u * 120:base + u * 120 + 120],
                            identb[:])

        def x2c_evict(b, ww):
            """ACT relu+bias evict of one 6-row tpw tile -> x2cat."""
            st = state[b]
            x2c = x2_pool.tile([120, 768], FP16, name="x2c_t", tag=f"x2c{ww}")
            nc.scalar.activation(x2c[:], st["tpw"][ww][:], AF.Relu,
                                 bias=b1_sb[:, 0:1])
            st["x2"].append(x2c)

        def conv2_group(b, g):
            """6 Toeplitz planes (edge planes N=160), pool2 reduce, T2."""
            st = state[b]
            x2cat = st["x2"]
            if g == 0:
                st["tp2"] = tp_pool.tile([80, 512], FP16, name="tp2w_t",
                                         tag="tp")
            tp2w = st["tp2"]
            ps2g = psx_pool.tile([128, 320], F32, name=f"ps2_{g}", tag="psx")

            def lhsT(r):
                return x2cat[r // 6][:, (r % 6) * 128:(r % 6 + 1) * 128]

            # d=0 runs full width [W0|Z] with start=True (start zeroes the
            # whole bank, so only ONE start per bank); d=5 edge accumulates
            # N=160 into the already-zeroed upper half
            nc.tensor.matmul(ps2g[:], lhsT(2 * g),
                             w2m_sb[:, 800:1120], start=True, stop=False,
                             skip_group_check=True)
            nc.tensor.matmul(ps2g[:, 160:320], lhsT(2 * g + 5),
                             w2m_sb[:, 160:320], start=False, stop=False,
                             skip_group_check=True)
            for d in range(1, 5):
                nc.tensor.matmul(ps2g[:], lhsT(2 * g + d),
                                 w2m_sb[:, (5 - d) * 160:(7 - d) * 160],
                                 start=False, stop=(d == 4),
                                 skip_group_check=True)
            p2 = p2_pool.tile([128, 80], FP16, name="p2_t", tag="p2")
            p2v = p2.rearrange("p (o s) -> p o s", o=20)
            src = ps2g.rearrange("p (pl o s tc) -> p o s pl tc",
                                 pl=2, o=20, s=4)
            nc.vector.reduce_max(p2v, src, axis=AX.XY)
            # defer the T2 transpose to the next group so the PE is not
            # blocked on this group's pool2 reduce
            if g > 0:
                nc.tensor.transpose(tp2w[:, (g - 1) * 128:g * 128],
                                    st["p2"], identb[:])
            st["p2"] = p2

        def f_evict(b):
            pair, half = b // 2, b % 2
            st = state[b]
            nc.tensor.transpose(st["tp2"][:, 384:512], st["p2"], identb[:])
            if half == 0:
                st["f"] = f_pool.tile([80, 1024], FP16, name="f_all",
                                      tag="f_all")
            else:
                st["f"] = state[b - 1]["f"]
            nc.scalar.activation(st["f"][:, half * 512:half * 512 + 512],
                                 st["tp2"][:], AF.Relu, bias=b2_sb[:, 0:1])

        def fc_pair(pair):
            """Batched fc1+fc2 for both blocks of a pair."""
            f_all = state[pair * 2 + 1]["f"]
            fview = f_all.rearrange("p (h g n) -> p g h n", h=2, g=4, n=128)
            psf1 = psx_pool.tile([50, 256], F32, name="psf1", tag="psx")
            for g in range(4):
                nc.tensor.matmul(psf1[:], wfc1_sb[:, g * 50:(g + 1) * 50],
                                 fview[:, g], start=(g == 0), stop=(g == 3))
            fc1o = fc1o_pool.tile([50, 256], FP16, name="fc1o", tag="fc1o")
            nc.scalar.activation(fc1o[:], psf1[:], AF.Relu,
                                 bias=bf1_sb[:, 0:1])
            for half in range(2):
                blk = pair * 2 + half
                psf2 = psx_pool.tile([128, 10], F32, name="psf2", tag="psx")
                nc.tensor.matmul(psf2[:],
                                 fc1o[:, half * 128:half * 128 + 128],
                                 wfc2_sb[:], start=True, stop=True)
                nc.vector.tensor_add(t1_all[:, blk * 10:blk * 10 + 10],
                                     psf2[:], bc2_sb[:])

        def fc_half(blk):
            """fc1+fc2 for one 128-sample block (independent sample half of
            the pair's f_all) - runs right after that block's f_evict."""
            half = blk % 2
            f_all = state[blk]["f"]
            fview = f_all.rearrange("p (h g n) -> p g h n", h=2, g=4, n=128)
            psf1 = psx_pool.tile([50, 128], F32, name="psf1", tag="psx")
            for g in range(4):
                nc.tensor.matmul(psf1[:], wfc1_sb[:, g * 50:(g + 1) * 50],
                                 fview[:, g, half], start=(g == 0),
                                 stop=(g == 3))
            fc1o = fc1o_pool.tile([50, 128], FP16, name="fc1o", tag="fc1o")
            nc.scalar.activation(fc1o[:], psf1[:], AF.Relu,
                                 bias=bf1_sb[:, 0:1])
            psf2 = psx_pool.tile([128, 10], F32, name="psf2", tag="psx")
            nc.tensor.matmul(psf2[:], fc1o[:], wfc2_sb[:],
                             start=True, stop=True)
            nc.vector.tensor_add(t1_all[:, blk * 10:blk * 10 + 10],
                                 psf2[:], bc2_sb[:])

        # ---------------- main software-pipelined issue loop --------------
        # iteration b: conv1 pairs of block b interleaved with conv2 groups
        # of block b-1 on the PE; ACT drains relu_d(b) between the x2cat
        # evicts of block b-1; DVE adds/maxes (b) between pool2 reduces of
        # b-1.  fc of pair p issues mid-iteration 2p+2.
        def epilogue_q(b0, nb, tag):
            """log_softmax for blocks b0 .. b0+nb-1 and their y DMA."""
            t1s = t1_all[:, b0 * 10:(b0 + nb) * 10]
            e4 = sm_pool.tile([128, 10 * nb], F32, name=f"e4_{tag}",
                              tag=f"e{tag}")
            nc.scalar.activation(e4[:], t1s, AF.Exp)
            se = sm_pool.tile([128, nb], F32, name=f"se_{tag}",
                              tag=f"se{tag}")
            nc.vector.reduce_sum(
                se[:], e4.rearrange("p (b t) -> p b t", t=10), axis=AX.X)
            ls = sm_pool.tile([128, nb], F32, name=f"ls_{tag}",
                              tag=f"ls{tag}")
            nc.scalar.activation(ls[:], se[:], AF.Ln)
            yo = sm_pool.tile([128, 10 * nb], F32, name=f"yo_{tag}",
                              tag=f"yo{tag}")
            for b in range(nb):
                nc.vector.tensor_scalar_sub(
                    yo[:, b * 10:b * 10 + 10],
                    t1s[:, b * 10:b * 10 + 10],
                    ls[:, b:b + 1])
            nc.sync.dma_start(
                y[b0 * 128:(b0 + nb) * 128]
                .rearrange("(blk p) c -> p blk c", p=128),
                yo.rearrange("p (blk c) -> p blk c", c=10))

        # iteration b issues conv1 pairs of block b; T1/conv2/evicts of
        # block b-1 slide into it as PE/ACT filler between the pairs.
        for b in range(N_BLK + 1):
            prev = b - 1
            if b < N_BLK:
                state[b] = {"prp": []}
                if b % 2 == 1 and b // 2 + 1 < n_pair:
                    issue_xw(b // 2 + 1)
                for w in range(6):
                    ps1p = conv1_pair(b, w)
                    if b == 0:
                        if w == 1:
                            nc.sync.dma_start(cst_sb[:], cst_d)
                        elif w == 2:
                            nc.sync.dma_start(w2m_sb[:], w2m_d)
                        elif w == 3:
                            nc.sync.dma_start(wfcb_sb[:], wfcb_d)

                    drain_pair(b, w, ps1p)
                    if prev >= 0:
                        if w == 1:
                            t1_transposes(prev)
                        elif w == 2:
                            x2c_evict(prev, 0)
                            conv2_group(prev, 0)
                        elif w == 3:
                            x2c_evict(prev, 1)
                            conv2_group(prev, 1)
                        elif w == 4:
                            conv2_group(prev, 2)
                        elif w == 5:
                            conv2_group(prev, 3)
                if prev >= 0:
                    f_evict(prev)
                    if b % 2 == 1 and b >= 3:
                        fc_pair((b - 3) // 2)

            else:
                fc_half(prev - 1)
                epilogue_q(0, 7, "a")
                t1_transposes(prev)
                x2c_evict(prev, 0)
                conv2_group(prev, 0)
                x2c_evict(prev, 1)
                for g in range(1, 4):
                    conv2_group(prev, g)
                f_evict(prev)
                fc_half(prev)
                epilogue_q(7, 1, "b")

    nc.compile()
    return nc


_PROGRAM_CACHE = {}


def _get_program(b_core):
    if b_core not in _PROGRAM_CACHE:
        _PROGRAM_CACHE[b_core] = _build(b_core)
    return _PROGRAM_CACHE[b_core]


def make_in_maps(x, weights, b_core=B_CORE, n_cores=N_CORES):
    """Shard x over cores; replicate the (rearranged) parameters."""
    f32 = np.float32
    xr = np.asarray(x, dtype=f32).reshape(-1, 28, 28)
    in_maps = []
    for c in range(n_cores):
        xc = xr[c * b_core:(c + 1) * b_core]  # [b_core, 28, 28]
        xwin = np.empty((12, 128, b_core), np.float16)
        for w in range(6):
            for h in range(2):
                win = xc[:, 4 * w:4 * w + 8, 12 * h:12 * h + 16]
                xwin[w * 2 + h] = win.reshape(b_core, 128).T
        # pair-major, partition-major: [pair][128][t*256+n]
        xp = xwin.reshape(12, 128, N_PAIR, 256).transpose(2, 1, 0, 3)
        m = {"xw": np.ascontiguousarray(xp.reshape(N_PAIR, 128, 3072))}
        m.update(weights)
        in_maps.append(m)
    return in_maps


def kernel(**inputs):
    x = np.asarray(inputs["x"], dtype=np.float32)
    weights = _prep_weights(
        np.asarray(inputs["mask_w"], np.float32),
        np.asarray(inputs["conv1_w"], np.float32),
        np.asarray(inputs["conv1_b"], np.float32),
        np.asarray(inputs["conv2_w"], np.float32),
        np.asarray(inputs["conv2_b"], np.float32),
        np.asarray(inputs["fc1_w"], np.float32),
        np.asarray(inputs["fc1_b"], np.float32),
        np.asarray(inputs["fc2_w"], np.float32),
        np.asarray(inputs["fc2_b"], np.float32),
    )
    nc = _get_program(B_CORE)
    in_maps = make_in_maps(x, weights)
    res = run_bass_kernel_spmd(nc, in_maps, list(range(N_CORES)))
    out = np.concatenate([res.results[c]["y"] for c in range(N_CORES)], axis=0)
    return np.ascontiguousarray(out.astype(np.float32))


if __name__ == "__main__":
    rng = np.random.default_rng(0)
    ins = {
        "x": rng.standard_normal((B_TOTAL, 1, 28, 28), dtype=np.float32),
        "mask_w": rng.standard_normal((28, 28), dtype=np.float32) * 0.1,
        "conv1_w": rng.standard_normal((10, 1, 5, 5), dtype=np.float32) * 0.2,
        "conv1_b": rng.standard_normal((10,), dtype=np.float32) * 0.1,
        "conv2_w": rng.standard_normal((20, 10, 5, 5), dtype=np.float32) * 0.06,
        "conv2_b": rng.standard_normal((20,), dtype=np.float32) * 0.1,
        "fc1_w": rng.standard_normal((50, 320), dtype=np.float32) * 0.05,
        "fc1_b": rng.standard_normal((50,), dtype=np.float32) * 0.1,
        "fc2_w": rng.standard_normal((10, 50), dtype=np.float32) * 0.14,
        "fc2_b": rng.standard_normal((10,), dtype=np.float32) * 0.1,
    }
    out = kernel(**ins)
    print(out.shape, out.dtype, out[:2])


# revision 76
# speedup vs baseline: 1.0082x; 1.0028x over previous
"""Trainium2 Bass kernel for nn_CNN_Casual (LeNet-ish CNN, B=8192).

Pure data parallel over 8 NeuronCores: 1024 samples per core, parameters
replicated, one SPMD Bass program. Per core, samples are processed in
blocks of 128 (the TensorEngine stationary-operand width).

v2 design notes (vs the first working version):

  conv1  : host gathers x into overlapping windows (8 input rows x 16 cols
           = K 128) and folds sigmoid(mask) into a per-window Toeplitz
           weight matrix (exact - the mask is elementwise on the input and
           conv is linear). Columns are [d-block | b-block]: for each
           pooled column pair (a, b) the PE emits d = a - b and b directly
           (differenced Toeplitz columns - same column count as the raw
           conv).  max(a, b) = relu(d) + b, so the 2x2 max-pool becomes:
             ACT: relu(d)  (PSUM -> fp16 SBUF, was a plain copy before)
             DVE: m = relu_d + b  (one PSUM operand - hw allows only one)
             DVE: prp = max(m_row0, m_row1)   (fp16, 2x mode)
           which costs ~half of the reduce/copy-based drain of v1.
  T1     : PE transposes prp [128,120] slices into tpw PSUM fp16; ACT
           evicts relu(x + b1) to x2cat (bias is per-partition there).
  conv2  : 6 input-row Toeplitz planes per output-row-pair group; the
           first plane runs full width over the zero-padded master block
           with start=True (a start zeroes the whole PSUM bank, so only
           one per bank), the opposite edge plane accumulates N=160 -
           1760 PE cols/group instead of 1920.
  pool2  : DVE 6D reduce_max per group (single PSUM operand), PE T2
           transposes, ACT relu+bias evict into f_all.
  fc1/fc2: weights stationary [80,50] x4 / data stationary [50,128].
  softmax: constant-shift (fc2_b - 10 folded host-side) then ONE batched
           epilogue at the end of the core (Exp, windowed reduce_sum, Ln,
           subs) - a single activation-table load set covers
           Relu/Copy/Exp/Ln so table reloads are minimized.
  warmup : a couple of identity transposes after the identity DMA start
           the cost-model p-state ramp so conv1 runs at full clock.
  sched  : one merged 3-slot x 2-bank PSUM pool (conv1 pairs + conv2
           groups + fc accumulators) + a 2-buf fp16 transpose pool; the
           issue loop slides T1/conv2/evicts of block b-1 between the
           conv1 pairs of block b; fc per pair at odd iterations; the
           bulk softmax epilogue overlaps the last block's conv2 and only
           block 7's 10 columns chain at the tail.

dtypes: conv inputs/weights fp16 (PSUM accumulation fp32), pooled
activations fp16.  relu(d)+b rounds once more than a direct max, adding
<= ~1e-3 relative error on pooled values; end-to-end max rel err vs the
fp32 reference stays ~1e-3 (gate is 2e-2).

DMA: input is shipped pair-major [pair][128][12*256] so each per-pair DMA
is 128 descriptors of 6KB; weights ~1.9MB once; one output DMA.
"""

from contextlib import ExitStack

import numpy as np

import concourse.mybir as mybir
import concourse.tile as tile
from concourse import bacc
from concourse.bass_utils import run_bass_kernel_spmd

F32 = mybir.dt.float32
FP16 = mybir.dt.float16
AF = mybir.ActivationFunctionType
AX = mybir.AxisListType

N_CORES = 8
B_TOTAL = 8192
B_CORE = B_TOTAL // N_CORES  # 1024
N_PAIR = B_CORE // 256       # 4
N_BLK = B_CORE // 128        # 8


# --------------------------------------------------------------------------
# Host-side weight preparation (tiny tensors; exact rearrangement only)
# --------------------------------------------------------------------------
def _prep_weights(mask_w, conv1_w, conv1_b, conv2_w, conv2_b, fc1_w, fc1_b,
                  fc2_w, fc2_b):
    f32 = np.float32
    sig = (1.0 / (1.0 + np.exp(-mask_w.astype(f32)))).astype(f32)  # [28,28]
    w1 = conv1_w.astype(f32)[:, 0]  # [10,5,5]

    # conv1 Toeplitz, windows (w,h): input rows 4w..4w+7, cols 12h..12h+15.
    # K-row = i*16+j.  Columns: [d-block 240 | b-block 240], each ordered
    # (u,tr,o,m): raw out row p = 4w+2u+tr, raw col pair q = 12h+2m+{0,1};
    # d = raw(q even) - raw(q odd), b = raw(q odd); mask folded per tap.
    w1m = np.zeros((12, 128, 480), f32)
    for w in range(6):
        for h in range(2):
            t = 2 * w + h
            for u in range(2):
                for tr in range(2):
                    p = 4 * w + 2 * u + tr
                    for m in range(6):
                        cd = u * 120 + tr * 60 + m          # + o*6
                        cb = 240 + cd
                        for ki in range(5):
                            i = 2 * u + tr + ki
                            for kj in range(5):
                                j0 = 2 * m + kj
                                j1 = 2 * m + 1 + kj
                                wv = w1[:, ki, kj]  # [10]
                                s0 = sig[p + ki, 12 * h + j0]
                                s1 = sig[p + ki, 12 * h + j1]
                                oo = np.arange(10) * 6
                                w1m[t, i * 16 + j1, cb + oo] += wv * s1
                                w1m[t, i * 16 + j0, cd + oo] += wv * s0
                                w1m[t, i * 16 + j1, cd + oo] -= wv * s1
    w1m = np.ascontiguousarray(
        w1m.transpose(1, 0, 2).reshape(128, 5760)).astype(np.float16)

    # conv2 master Toeplitz: blocks [Z, W4, W3, W2, W1, W0, Z], each [120,160]
    # row index (c, j) = c*12 + j; col index (o2, q2) = o2*8 + q2
    w2m = np.zeros((120, 7, 160), np.float16)
    o2 = np.arange(20)
    for k in range(5):
        blk = 5 - k
        for c in range(10):
            for kj in range(5):
                for q2 in range(8):
                    j = q2 + kj
                    w2m[c * 12 + j, blk, o2 * 8 + q2] = conv2_w[:, c, k, kj]
    w2m_flat = np.ascontiguousarray(w2m.reshape(120, 7 * 160))
    # permute conv2 input rows from (c, j=h*6+m) to (h, c, m) so the T1
    # transpose reads contiguous prp2 blocks (see drain_pair)
    perm = np.empty(120, np.int64)
    for c in range(10):
        for h in range(2):
            for m in range(6):
                perm[h * 60 + c * 6 + m] = c * 12 + h * 6 + m
    w2m_flat = np.ascontiguousarray(w2m_flat[perm])

    # fc1 weights per pooled-row group p': rows (o2, s2), torch flatten order
    # of the conv2 activations is (o2, p', s2).
    fc1w4 = fc1_w.reshape(50, 20, 4, 4)  # [m, o2, p', s2]
    wfc1 = np.concatenate(
        [np.ascontiguousarray(fc1w4[:, :, p, :].reshape(50, 80).T)
         for p in range(4)],
        axis=1,
    )  # [80, 200]

    # const blob 1 (fp32): ident | bc2 | b1 | b2 | bf1  -> [128, 141]
    cst = np.zeros((128, 141), f32)
    cst[:, 0:128] = np.eye(128, dtype=f32)
    # constant stabilizing shift for log_softmax (exact: any per-sample
    # constant cancels); logits stay well inside fp32 exp range
    cst[:, 128:138] = np.tile(fc2_b.astype(f32).reshape(1, 10) - 10.0,
                              (128, 1))
    cst[0:120, 138] = np.tile(np.repeat(conv1_b.astype(f32), 6), 2)
    cst[0:80, 139] = np.repeat(conv2_b.astype(f32), 4)
    cst[0:50, 140] = fc1_b.astype(f32)

    # const blob 2 (fp16): fc2_w.T | wfc1 -> [80, 210]
    wfcb = np.zeros((80, 210), np.float16)
    wfcb[0:50, 0:10] = fc2_w.T.astype(np.float16)
    wfcb[:, 10:210] = wfc1.astype(np.float16)

    idb = np.eye(128).astype(np.float16)
    return dict(w1m=w1m, w2m=w2m_flat, wfcb=wfcb, cst=cst, idb=idb)


# --------------------------------------------------------------------------
# Device program
# --------------------------------------------------------------------------
def _build(b_core):
    assert b_core == 1024
    n_pair = N_PAIR

    nc = bacc.Bacc("TRN2", target_bir_lowering=False, debug=False,
                   num_devices=N_CORES)

    xw_d = nc.dram_tensor("xw", [n_pair, 128, 3072], FP16,
                          kind="ExternalInput").ap()
    w1m_d = nc.dram_tensor("w1m", [128, 5760], FP16,
                           kind="ExternalInput").ap()
    w2m_d = nc.dram_tensor("w2m", [120, 1120], FP16, kind="ExternalInput").ap()
    wfcb_d = nc.dram_tensor("wfcb", [80, 210], FP16, kind="ExternalInput").ap()
    cst_d = nc.dram_tensor("cst", [128, 141], F32, kind="ExternalInput").ap()
    idb_d = nc.dram_tensor("idb", [128, 128], FP16, kind="ExternalInput").ap()
    y = nc.dram_tensor("y", [b_core, 10], F32, kind="ExternalOutput").ap()

    with tile.TileContext(nc) as tc, ExitStack() as ctx:
        consts = ctx.enter_context(tc.tile_pool(name="consts", bufs=1))
        identb = consts.tile([128, 128], FP16)
        w1m_sb = consts.tile([128, 5760], FP16)
        w2m_sb = consts.tile([120, 1120], FP16)
        wfcb_sb = consts.tile([80, 210], FP16)
        cst_sb = consts.tile([128, 141], F32)
        t1_all = consts.tile([128, 10 * N_BLK], F32)

        bc2_sb = cst_sb[:, 128:138]
        b1_sb = cst_sb[0:120, 138:139]
        b2_sb = cst_sb[0:80, 139:140]
        bf1_sb = cst_sb[0:50, 140:141]
        wfc2_sb = wfcb_sb[0:50, 0:10]
        wfc1_sb = wfcb_sb[:, 10:210]

        xw_pool = ctx.enter_context(tc.tile_pool(name="xw", bufs=2))
        # one merged 3-slot x 2-bank PSUM pool for conv1 pairs, conv2
        # groups, and fc accumulators; tp (fp16 transpose staging) separate
        psx_pool = ctx.enter_context(tc.tile_pool(name="psx", bufs=3,
                                                  space="PSUM"))
        rd_pool = ctx.enter_context(tc.tile_pool(name="rd", bufs=6))
        mp_pool = ctx.enter_context(tc.tile_pool(name="mp", bufs=6))
        prp_pool = ctx.enter_context(tc.tile_pool(name="prp", bufs=14))
        tp_pool = ctx.enter_context(tc.tile_pool(name="tp", bufs=2,
                                                 space="PSUM"))
        x2_pool = ctx.enter_context(tc.tile_pool(name="x2", bufs=6))
        p2_pool = ctx.enter_context(tc.tile_pool(name="p2", bufs=6))
        f_pool = ctx.enter_context(tc.tile_pool(name="fp", bufs=2))
        fc1o_pool = ctx.enter_context(tc.tile_pool(name="fc1o", bufs=2))
        sm_pool = ctx.enter_context(tc.tile_pool(name="sm", bufs=1))

        zeros_sb = consts.tile([120, 768], FP16)
        nc.vector.memset(zeros_sb[:], 0.0)

        # ---- PE warmup: ramp the p-state while DMAs land ----
        warm = psx_pool.tile([128, 128], FP16, name="warm", tag="psx")
        for _ in range(4):
            nc.tensor.transpose(warm[:], identb[:], identb[:])
        # pre-load the activation-table set covering Relu/Copy/Exp/Ln so the
        # fixpoint pass inserts no further loads (set 6 =
        # natural_log_exp_and_others in act_info.json insertion order)
        nc.scalar.add_instruction(mybir.InstLoadActFuncSet(
            name=nc.get_next_instruction_name(), act_func_set_id=6,
            ins=[], outs=[]))

        # initial weight DMAs: xw on SP/HWDGE, weights on gpsimd/SWDGE so
        # the two issue paths run in parallel at startup
        xw_tiles = [None] * n_pair

        def issue_xw(pair, defer=False):
            xwc = xw_pool.tile([128, 3072], FP16, name=f"xwc{pair}", tag="xw")
            xw_tiles[pair] = xwc
            if not defer:
                nc.sync.dma_start(xwc[:], xw_d[pair])
            return xwc

        xw_tiles_first = issue_xw(0, defer=True)

        nc.sync.dma_start(xw_tiles_first[:, 0:1536], xw_d[0, :, 0:1536])
        nc.sync.dma_start(identb[:], idb_d)
        for q in range(3):
            nc.sync.dma_start(w1m_sb[:, q * 960:(q + 1) * 960],
                              w1m_d[:, q * 960:(q + 1) * 960])
        nc.sync.dma_start(xw_tiles_first[:, 1536:3072], xw_d[0, :, 1536:3072])
        for q in range(3, 6):
            nc.sync.dma_start(w1m_sb[:, q * 960:(q + 1) * 960],
                              w1m_d[:, q * 960:(q + 1) * 960])

        # per-block state
        state = {}

        def conv1_pair(b, w):
            """Two window matmuls (t=2w, 2w+1) into one 2-bank PSUM tile.

            High priority: the PE should always prefer feeding the drain
            pipeline; conv2/transposes of the previous block fill the gaps.
            """
            pair, half = b // 2, b % 2
            ps1p = psx_pool.tile([128, 1024], F32, name="ps1_t", tag="psx")
            with tc.high_priority(offset=30):
                for h in range(2):
                    t = 2 * w + h
                    nc.tensor.matmul(ps1p[:, h * 512:h * 512 + 480],
                                     xw_tiles[pair][:, t * 256 + half * 128:
                                                    t * 256 + half * 128
                                                    + 128],
                                     w1m_sb[:, t * 480:(t + 1) * 480],
                                     start=True, stop=True)
            return ps1p

        def drain_pair(b, w, ps1p):
            """relu(d) on ACT, += b on DVE; level-2 max batched per two
            window-pairs into a [128, 480] prp2 tile."""
            st = state[b]
            psv = ps1p.rearrange("p (h q) -> p h q", h=2)
            rd = rd_pool.tile([128, 480], FP16, name="rd_t", tag="rd")
            if w % 2 == 0:
                st["mp"] = mp_pool.tile([128, 960], FP16, name="mp_t",
                                        tag="mp")
            m_sup = st["mp"]
            # m_sup layout (P, u, h, tr, o, m): the level-2 group dim
            # (P,u,h) then matches prp2's contiguous (P,u,h,o,m) layout
            m_pair = m_sup[:, (w % 2) * 480:(w % 2) * 480 + 480]
            nc.scalar.activation(rd.rearrange("p (h c) -> p h c", h=2),
                                 psv[:, :, 0:240], AF.Relu)
            nc.vector.tensor_add(
                m_pair.rearrange("p (u h c) -> p h u c", u=2, h=2),
                rd.rearrange("p (h u c) -> p h u c", h=2, u=2),
                psv[:, :, 240:480].rearrange("p h (u c) -> p h u c", u=2))
            if w % 2 == 1:
                # prp2 contiguous layout (P, h, u, o, m): both tensor_max
                # APs merge to <=2 free dims (walrus limit); the T1
                # transpose reads a strided (o, h, m) view instead.
                prp2 = prp_pool.tile([128, 480], FP16, name="prp_t",
                                     tag="prp")
                st["prp"].append(prp2)
                mv = m_sup.rearrange("p (g tr om) -> p g tr om",
                                     g=8, tr=2)
                pv = prp2.rearrange("p (g om) -> p g om", g=8)
                nc.vector.tensor_max(pv, mv[:, :, 0], mv[:, :, 1])

        def t1_transposes(b):
            """PE transposes prp2 slices -> two 6-row tpw tiles [120, 768].

            prp2 tile P holds windows 2P (cols 0:240) and 2P+1 (240:480).
            """
            st = state[b]
            st["tpw"] = []
            st["x2"] = []
            for ww in range(2):
                tpw = tp_pool.tile([120, 768], FP16, name="tpw_t", tag="tp")
                st["tpw"].append(tpw)
                for k in range(3):
                    w = ww * 3 + k
                    prp2 = st["prp"][w // 2]
                    base = (w % 2) * 240
                    for u in range(2):
                        nc.tensor.transpose(
                            tpw[:, (k * 2 + u) * 128:(k * 2 + u + 1) * 128],
                            prp2[:, base + u * 120:base + u * 120 + 120],
                            identb[:])

        def x2c_evict(b, ww):
            """ACT relu+bias evict of one 6-row tpw tile -> x2cat."""
            st = state[b]
            x2c = x2_pool.tile([120, 768], FP16, name="x2c_t", tag=f"x2c{ww}")
            nc.scalar.activation(x2c[:], st["tpw"][ww][:], AF.Relu,
                                 bias=b1_sb[:, 0:1])
            st["x2"].append(x2c)

        def conv2_group(b, g):
            """6 Toeplitz planes (edge planes N=160), pool2 reduce, T2."""
            st = state[b]
            x2cat = st["x2"]
            if g == 0:
                st["tp2"] = tp_pool.tile([80, 512], FP16, name="tp2w_t",
                                         tag="tp")
            tp2w = st["tp2"]
            ps2g = psx_pool.tile([128, 320], F32, name=f"ps2_{g}", tag="psx")

            def lhsT(r):
                return x2cat[r // 6][:, (r % 6) * 128:(r % 6 + 1) * 128]

            # d=0 runs full width [W0|Z] with start=True (start zeroes the
            # whole bank, so only ONE start per bank); d=5 edge accumulates
            # N=160 into the already-zeroed upper half
            nc.tensor.matmul(ps2g[:], lhsT(2 * g),
                             w2m_sb[:, 800:1120], start=True, stop=False,
                             skip_group_check=True)
            nc.tensor.matmul(ps2g[:, 160:320], lhsT(2 * g + 5),
                             w2m_sb[:, 160:320], start=False, stop=False,
                             skip_group_check=True)
            for d in range(1, 5):
                nc.tensor.matmul(ps2g[:], lhsT(2 * g + d),
                                 w2m_sb[:, (5 - d) * 160:(7 - d) * 160],
                                 start=False, stop=(d == 4),
                                 skip_group_check=True)
            p2 = p2_pool.tile([128, 80], FP16, name="p2_t", tag="p2")
            p2v = p2.rearrange("p (o s) -> p o s", o=20)
            src = ps2g.rearrange("p (pl o s tc) -> p o s pl tc",
                                 pl=2, o=20, s=4)
            nc.vector.reduce_max(p2v, src, axis=AX.XY)
            # defer the T2 transpose to the next group so the PE is not
            # blocked on this group's pool2 reduce
            if g > 0:
                nc.tensor.transpose(tp2w[:, (g - 1) * 128:g * 128],
                                    st["p2"], identb[:])
            st["p2"] = p2

        def f_evict(b):
            pair, half = b // 2, b % 2
            st = state[b]
            nc.tensor.transpose(st["tp2"][:, 384:512], st["p2"], identb[:])
            if half == 0:
                st["f"] = f_pool.tile([80, 1024], FP16, name="f_all",
                                      tag="f_all")
            else:
                st["f"] = state[b - 1]["f"]
            nc.scalar.activation(st["f"][:, half * 512:half * 512 + 512],
                                 st["tp2"][:], AF.Relu, bias=b2_sb[:, 0:1])

        def fc_pair(pair):
            """Batched fc1+fc2 for both blocks of a pair."""
            f_all = state[pair * 2 + 1]["f"]
            fview = f_all.rearrange("p (h g n) -> p g h n", h=2, g=4, n=128)
            psf1 = psx_pool.tile([50, 256], F32, name="psf1", tag="psx")
            for g in range(4):
                nc.tensor.matmul(psf1[:], wfc1_sb[:, g * 50:(g + 1) * 50],
                                 fview[:, g], start=(g == 0), stop=(g == 3))
            fc1o = fc1o_pool.tile([50, 256], FP16, name="fc1o", tag="fc1o")
            nc.scalar.activation(fc1o[:], psf1[:], AF.Relu,
                                 bias=bf1_sb[:, 0:1])
            for half in range(2):
                blk = pair * 2 + half
                psf2 = psx_pool.tile([128, 10], F32, name="psf2", tag="psx")
                nc.tensor.matmul(psf2[:],
                                 fc1o[:, half * 128:half * 128 + 128],
                                 wfc2_sb[:], start=True, stop=True)
                nc.vector.tensor_add(t1_all[:, blk * 10:blk * 10 + 10],
                                     psf2[:], bc2_sb[:])

        def fc_half(blk):
            """fc1+fc2 for one 128-sample block (independent sample half of
            the pair's f_all) - runs right after that block's f_evict."""
            half = blk % 2
            f_all = state[blk]["f"]
            fview = f_all.rearrange("p (h g n) -> p g h n", h=2, g=4, n=128)
            psf1 = psx_pool.tile([50, 128], F32, name="psf1", tag="psx")
            for g in range(4):
                nc.tensor.matmul(psf1[:], wfc1_sb[:, g * 50:(g + 1) * 50],
                                 fview[:, g, half], start=(g == 0),
                                 stop=(g == 3))
            fc1o = fc1o_pool.tile([50, 128], FP16, name="fc1o", tag="fc1o")
            nc.scalar.activation(fc1o[:], psf1[:], AF.Relu,
                                 bias=bf1_sb[:, 0:1])
            psf2 = psx_pool.tile([128, 10], F32, name="psf2", tag="psx")
            nc.tensor.matmul(psf2[:], fc1o[:], wfc2_sb[:],
                             start=True, stop=True)
            nc.vector.tensor_add(t1_all[:, blk * 10:blk * 10 + 10],
                                 psf2[:], bc2_sb[:])

        # ---------------- main software-pipelined issue loop --------------
        # iteration b: conv1 pairs of block b interleaved with conv2 groups
        # of block b-1 on the PE; ACT drains relu_d(b) between the x2cat
        # evicts of block b-1; DVE adds/maxes (b) between pool2 reduces of
        # b-1.  fc of pair p issues mid-iteration 2p+2.
        def epilogue_q(b0, nb, tag):
            """log_softmax for blocks b0 .. b0+nb-1 and their y DMA."""
            t1s = t1_all[:, b0 * 10:(b0 + nb) * 10]
            e4 = sm_pool.tile([128, 10 * nb], F32, name=f"e4_{tag}",
                              tag=f"e{tag}")
            nc.scalar.activation(e4[:], t1s, AF.Exp)
            se = sm_pool.tile([128, nb], F32, name=f"se_{tag}",
                              tag=f"se{tag}")
            nc.vector.reduce_sum(
                se[:], e4.rearrange("p (b t) -> p b t", t=10), axis=AX.X)
            ls = sm_pool.tile([128, nb], F32, name=f"ls_{tag}",
                              tag=f"ls{tag}")
            nc.scalar.activation(ls[:], se[:], AF.Ln)
            yo = sm_pool.tile([128, 10 * nb], F32, name=f"yo_{tag}",
                              tag=f"yo{tag}")
            for b in range(nb):
                nc.vector.tensor_scalar_sub(
                    yo[:, b * 10:b * 10 + 10],
                    t1s[:, b * 10:b * 10 + 10],
                    ls[:, b:b + 1])
            nc.sync.dma_start(
                y[b0 * 128:(b0 + nb) * 128]
                .rearrange("(blk p) c -> p blk c", p=128),
                yo.rearrange("p (blk c) -> p blk c", c=10))

        # iteration b issues conv1 pairs of block b; T1/conv2/evicts of
        # block b-1 slide into it as PE/ACT filler between the pairs.
        for b in range(N_BLK + 1):
            prev = b - 1
            if b < N_BLK:
                state[b] = {"prp": []}
                if b % 2 == 1 and b // 2 + 1 < n_pair:
                    issue_xw(b // 2 + 1)
                for w in range(6):
                    ps1p = conv1_pair(b, w)
                    if b == 0:
                        if w == 1:
                            nc.sync.dma_start(cst_sb[:], cst_d)
                        elif w == 2:
                            nc.sync.dma_start(w2m_sb[:], w2m_d)
                        elif w == 3:
                            nc.sync.dma_start(wfcb_sb[:], wfcb_d)

                    drain_pair(b, w, ps1p)
                    if prev >= 0:
                        if w == 1:
                            t1_transposes(prev)
                        elif w == 2:
                            x2c_evict(prev, 0)
                            conv2_group(prev, 0)
                        elif w == 3:
                            x2c_evict(prev, 1)
                            conv2_group(prev, 1)
                        elif w == 4:
                            conv2_group(prev, 2)
                        elif w == 5:
                            conv2_group(prev, 3)
                if prev >= 0:
                    f_evict(prev)
                    if b % 2 == 1 and b >= 3:
                        fc_pair((b - 3) // 2)

            else:
                fc_half(prev - 1)
                epilogue_q(0, 7, "a")
                t1_transposes(prev)
                x2c_evict(prev, 0)
                st7 = state[prev]
                x2c_d = x2_pool.tile([120, 768], FP16, name="x2c_t",
                                     tag="x2c1")
                nc.vector.scalar_tensor_tensor(
                    x2c_d[:], st7["tpw"][1][:], b1_sb[:, 0:1], zeros_sb[:],
                    op0=mybir.AluOpType.add, op1=mybir.AluOpType.max)
                st7["x2"].append(x2c_d)
                conv2_group(prev, 0)
                for g in range(1, 4):
                    conv2_group(prev, g)
                f_evict(prev)
                fc_half(prev)
                epilogue_q(7, 1, "b")

    nc.compile()
    return nc


_PROGRAM_CACHE = {}


def _get_program(b_core):
    if b_core not in _PROGRAM_CACHE:
        _PROGRAM_CACHE[b_core] = _build(b_core)
    return _PROGRAM_CACHE[b_core]


def make_in_maps(x, weights, b_core=B_CORE, n_cores=N_CORES):
    """Shard x over cores; replicate the (rearranged) parameters."""
    f32 = np.float32
    xr = np.asarray(x, dtype=f32).reshape(-1, 28, 28)
    in_maps = []
    for c in range(n_cores):
        xc = xr[c * b_core:(c + 1) * b_core]  # [b_core, 28, 28]
        xwin = np.empty((12, 128, b_core), np.float16)
        for w in range(6):
            for h in range(2):
                win = xc[:, 4 * w:4 * w + 8, 12 * h:12 * h + 16]
                xwin[w * 2 + h] = win.reshape(b_core, 128).T
        # pair-major, partition-major: [pair][128][t*256+n]
        xp = xwin.reshape(12, 128, N_PAIR, 256).transpose(2, 1, 0, 3)
        m = {"xw": np.ascontiguousarray(xp.reshape(N_PAIR, 128, 3072))}
        m.update(weights)
        in_maps.append(m)
    return in_maps


def kernel(**inputs):
    x = np.asarray(inputs["x"], dtype=np.float32)
    weights = _prep_weights(
        np.asarray(inputs["mask_w"], np.float32),
        np.asarray(inputs["conv1_w"], np.float32),
        np.asarray(inputs["conv1_b"], np.float32),
        np.asarray(inputs["conv2_w"], np.float32),
        np.asarray(inputs["conv2_b"], np.float32),
        np.asarray(inputs["fc1_w"], np.float32),
        np.asarray(inputs["fc1_b"], np.float32),
        np.asarray(inputs["fc2_w"], np.float32),
        np.asarray(inputs["fc2_b"], np.float32),
    )
    nc = _get_program(B_CORE)
    in_maps = make_in_maps(x, weights)
    res = run_bass_kernel_spmd(nc, in_maps, list(range(N_CORES)))
    out = np.concatenate([res.results[c]["y"] for c in range(N_CORES)], axis=0)
    return np.ascontiguousarray(out.astype(np.float32))


if __name__ == "__main__":
    rng = np.random.default_rng(0)
    ins = {
        "x": rng.standard_normal((B_TOTAL, 1, 28, 28), dtype=np.float32),
        "mask_w": rng.standard_normal((28, 28), dtype=np.float32) * 0.1,
        "conv1_w": rng.standard_normal((10, 1, 5, 5), dtype=np.float32) * 0.2,
        "conv1_b": rng.standard_normal((10,), dtype=np.float32) * 0.1,
        "conv2_w": rng.standard_normal((20, 10, 5, 5), dtype=np.float32) * 0.06,
        "conv2_b": rng.standard_normal((20,), dtype=np.float32) * 0.1,
        "fc1_w": rng.standard_normal((50, 320), dtype=np.float32) * 0.05,
        "fc1_b": rng.standard_normal((50,), dtype=np.float32) * 0.1,
        "fc2_w": rng.standard_normal((10, 50), dtype=np.float32) * 0.14,
        "fc2_b": rng.standard_normal((10,), dtype=np.float32) * 0.1,
    }
    out = kernel(**ins)
    print(out.shape, out.dtype, out[:2])
